# revision 32
# baseline (speedup 1.0000x reference)
"""Multi-head attention (B=2, S=2048, RES=1024, H=16) on 8 NeuronCores.

Sharding: batch*heads across cores. Core c handles batch c//4 and heads
4*(c%4) .. 4*(c%4)+3 (column-sharded QKV weights). No cross-core comm.

Pipeline design (vs the naive phase-by-phase version):
  - Host pre-packs x into the transposed SBUF layout and all 24 weight
    k-chunks into one blob, so the input lands as 256 large descriptors
    (~18us at HBM bandwidth) instead of 4600 small ones.
  - Prologue projects QT/KT for heads 0,1 only (d-rows duplicated into
    partitions 64:128 so the two 512-wide QK matmuls of a score tile
    run as concurrent PE row tiles); first exp fires ~20us in.
  - Main loop over segments (head, s-half) x t-pairs: QK scores (PSUM)
    -> exp on ACT (bf16, scale=1/8) -> PV (full-K bf16, M=66 with a
    ones column so softmax sums ride along).  PV lags the exp stream by
    LAG iterations, tapering to 2 once background work drains.
  - V projection and heads 2,3 QT/KT projection are "ensure" units:
    consumers force-emit their producers, so any pacing of the
    background pump is correct by construction (Tile engines execute in
    emission order - a consumer emitted before its producer silently
    reads stale data).
  - Per-(head,s-half) epilogue (PSUM drain, xbar detranspose in two
    halves, 1/sums scale) is inline; out DMAs for head-columns 0:192
    ship one segment early so the tail only writes the last 64 columns.
PE (projections + QK + PV, ~150us) and ACT (16.8M exps, ~131-160us
clock-dependent) are both near-critical; PSUM: 2 score tiles (4 banks)
+ PV accumulator (2) + projection staging (2).
"""

import sys

if "/opt/trn_rl_repo" not in sys.path:
    sys.path.insert(0, "/opt/trn_rl_repo")

import numpy as np

B = 2
S = 2048
RES = 1024
HEADS = 16
HD = 64
N_CORES = 8
HPC = 4
C = HPC * HD  # 256
K = RES
NKT = K // 128  # 8
NST = S // 128  # 16 t/s blocks
NTP = NST // 2  # 8 t-pairs
SH = 1024  # s-half
VW = 80  # oT row count for xbar transpose (64 d + ones + pad)
VAUG = 66  # per-head width in v_aug (64 d + ones col + pad)
LAG = 10  # PV lag in (head, s-half, t-pair) iterations

_CACHE: dict = {}


def _build_nc():
    import concourse.mybir as mybir
    import concourse.tile as tile
    from concourse import bacc

    f32 = mybir.dt.float32
    bf16 = mybir.dt.bfloat16
    AF = mybir.ActivationFunctionType

    nc = bacc.Bacc(None)
    x_in = nc.dram_tensor("x", [128, NKT * S], bf16, kind="ExternalInput")  # host-packed xT layout
    w_in = nc.dram_tensor("w", [128, 3 * NKT * C], bf16, kind="ExternalInput")  # host-packed wq|wk|wv chunks
    out_d = nc.dram_tensor("out", [S, C], f32, kind="ExternalOutput")

    with tile.TileContext(nc) as tc:
        with (
            tc.tile_pool(name="persist", bufs=1) as persist,
            tc.tile_pool(name="flow", bufs=2) as flow,
            tc.tile_pool(name="ps", bufs=1, space="PSUM") as ps,
        ):
            zeros128 = persist.tile([128, 128], bf16)
            nc.vector.memset(zeros128[:], 0.0)

            # ACT table preload: tiny exp early so the ~2.7us table load
            # overlaps the initial DMA wait
            warmup_exp = persist.tile([1, 16], bf16)
            nc.scalar.activation(
                warmup_exp[:], zeros128[0:1, 0:16], AF.Exp, scale=0.125
            )

            # ---- input DMAs (one packed weight blob: 128 x 12KB
            # descriptors instead of 3072 x 512B; xT right behind) ----
            wqk_sb = persist.tile([128, 2 * NKT, C], bf16, name="wqk_sb")
            nc.gpsimd.dma_start(
                wqk_sb.rearrange("p a b -> p (a b)"), w_in[:, 0 : 2 * NKT * C]
            )
            wv_sb = persist.tile([128, NKT, C], bf16, name="wv_sb")
            nc.gpsimd.dma_start(
                wv_sb.rearrange("p a b -> p (a b)"), w_in[:, 2 * NKT * C :]
            )
            wq_t = [wqk_sb[:, kk, :] for kk in range(NKT)]
            wk_t = [wqk_sb[:, NKT + kk, :] for kk in range(NKT)]
            wv_t = [wv_sb[:, kk, :] for kk in range(NKT)]
            # two xT tiles (k-chunks 0-3 / 4-7): Tile deps are per-tile,
            # so first-half projections start as soon as half 0 lands
            HK = NKT // 2
            xTa = persist.tile([128, HK * S], bf16, name="xTa")
            xTb = persist.tile([128, HK * S], bf16, name="xTb")
            nc.sync.dma_start(xTa[:], x_in[:, 0 : HK * S])
            nc.sync.dma_start(xTb[:], x_in[:, HK * S : NKT * S])
            xTa3 = xTa.rearrange("p (k s) -> p k s", k=HK)
            xTb3 = xTb.rearrange("p (k s) -> p k s", k=HK)

            def xchunk(kk, sl):
                if kk < HK:
                    return xTa3[:, kk, sl]
                return xTb3[:, kk - HK, sl]

            # per-head Q^T/K^T, d-rows duplicated into partitions 64:128
            qt_tiles = [
                persist.tile([128, S], bf16, name=f"qt_{h}", tag="qt", bufs=HPC)
                for h in range(HPC)
            ]
            kt_tiles = [
                persist.tile([128, S], bf16, name=f"kt_{h}", tag="kt", bufs=HPC)
                for h in range(HPC)
            ]

            # bf16 V tiles per t-block: [128, 4*VAUG]; per head: V cols
            # 0:64, ones at 64, zero pad at 65
            v_aug = []
            for st in range(NST):
                va = persist.tile(
                    [128, HPC * VAUG], bf16, name=f"va_{st}", tag="va", bufs=NST
                )
                nc.vector.memset(va[:], 0.0)
                nc.vector.memset(
                    va.rearrange("p (h w) -> p h w", h=HPC)[:, :, HD : HD + 1],
                    1.0,
                )
                v_aug.append(va)

            out_tiles = [
                persist.tile([128, C], f32, name=f"out_{sb}", tag="ot", bufs=NST)
                for sb in range(NST)
            ]

            # PSUM: sc 2x[128,1024] (4 banks) + outp 1x[80,1024] (2 banks)
            # + pp 2x[128,512] (2 banks) = 8 banks
            pp_warm = ps.tile([128, 512], f32, name="pp_warm", tag="pp", bufs=2)
            for w in range(24):
                nc.tensor.matmul(
                    pp_warm[:, 0:128],
                    zeros128[:],
                    zeros128[:],
                    start=True,
                    stop=True,
                    skip_group_check=True,
                )

            # ---------- background work units (PE-stream filler) ----------
            proj_qk_store = {}

            def proj_qk_unit(w_t, dsts, hp, sc, half):
                # Row-tiled halves (k rows 0:64 / 64:128 on PE row groups
                # 0/64) run concurrently and hide each other's LDWEIGHTS;
                # DVE sums the two partial products at staging time.
                def emit():
                    key = f"{id(dsts)}_{hp}_{sc}"
                    if half == 0:
                        pp = ps.tile(
                            [128, 512], f32, name=f"pp_{key}", tag="pp", bufs=2
                        )
                        proj_qk_store[key] = pp
                    else:
                        pp = proj_qk_store.pop(key)
                    for kk in range(half * 4, half * 4 + 4):
                        nc.tensor.matmul(
                            pp[:],
                            w_t[kk][:, hp * 128 : (hp + 1) * 128],
                            xchunk(kk, slice(sc * 512, (sc + 1) * 512)),
                            start=(kk == 0),
                            stop=(kk == NKT - 1),
                        )
                    if half == 0:
                        return
                    stg = flow.tile(
                        [128, 512], bf16, name=f"stg_{key}", tag="stg", bufs=2
                    )
                    nc.vector.tensor_copy(stg[:], pp[:])
                    cols = slice(sc * 512, (sc + 1) * 512)
                    for hh in range(2):
                        h = hp * 2 + hh
                        nc.vector.tensor_copy(
                            dsts[h][0:HD, cols], stg[hh * HD : (hh + 1) * HD, :]
                        )
                        nc.vector.tensor_copy(
                            dsts[h][HD:128, cols], stg[hh * HD : (hh + 1) * HD, :]
                        )
                return emit

            def vproj_unit(st, half):
                # V projection for s-block st, row-tiled halves like above;
                # DVE sums partials straight into the strided v_aug layout
                def emit():
                    key = f"v_{st}"
                    if half == 0:
                        vp = ps.tile(
                            [128, 512], f32, name=f"vp_{st}", tag="pp", bufs=2
                        )
                        proj_qk_store[key] = vp
                    else:
                        vp = proj_qk_store.pop(key)
                    for kk in range(half * 4, half * 4 + 4):
                        nc.tensor.matmul(
                            vp[:, 0:C],
                            xchunk(kk, slice(st * 128, (st + 1) * 128)),
                            wv_t[kk],
                            start=(kk == 0),
                            stop=(kk == NKT - 1),
                        )
                    if half == 0:
                        return
                    dst = v_aug[st].rearrange("p (h w) -> p h w", h=HPC)[
                        :, :, 0:HD
                    ]
                    nc.vector.tensor_copy(
                        dst, vp[:, 0:C].rearrange("p (h d) -> p h d", h=HPC)
                    )
                return emit

            # Unit registry: consumers force-emit ("ensure") their producer
            # units, so emission order is correct by construction no matter
            # how the background pump is paced.
            units = {}
            for hp in range(2):
                for sc in range(4):
                    units[("q", hp, sc)] = [
                        proj_qk_unit(wq_t, qt_tiles, hp, sc, half)
                        for half in range(2)
                    ]
                    units[("k", hp, sc)] = [
                        proj_qk_unit(wk_t, kt_tiles, hp, sc, half)
                        for half in range(2)
                    ]
            for st in range(NST):
                units[("v", st)] = [vproj_unit(st, half) for half in range(2)]
            emitted = set()

            def ensure(key):
                if key in emitted:
                    return
                emitted.add(key)
                for fn in units[key]:
                    fn()

            # background order, in deadline order
            bg_order = (
                [("k", 0, sc) for sc in range(4)]
                + [("q", 0, 2), ("q", 0, 3)]
                + [("v", st) for st in range(NST)]
                + [("q", 1, 0), ("q", 1, 1)]
                + [("k", 1, sc) for sc in range(4)]
                + [("q", 1, 2), ("q", 1, 3)]
                + [("q", 0, 0), ("q", 0, 1)]
            )
            bg_pos = [0]

            def pump():
                while bg_pos[0] < len(bg_order):
                    key = bg_order[bg_pos[0]]
                    if key not in emitted:
                        ensure(key)
                        return True
                    bg_pos[0] += 1
                return False

            def bg_pending():
                return any(k not in emitted for k in bg_order)

            # prologue: what iteration 0 needs. Emit the xTa-gated first
            # halves of q00/k00 before any xTb-gated second half, so the
            # in-order PE queue starts as soon as the first 2MB of x lands
            # (pp tag has exactly 2 buffers: only two units open at once).
            for key in (("q", 0, 0), ("k", 0, 0)):
                emitted.add(key)
                units[key][0]()
            for key in (("q", 0, 0), ("k", 0, 0)):
                units[key][1]()
            ensure(("q", 0, 1))

            # ---------- main attention pipeline ----------
            segs = [(0, 0), (1, 0), (0, 1), (1, 1),
                    (2, 0), (3, 0), (2, 1), (3, 1)]
            n_iter = len(segs) * NTP  # 64
            NAT = 2 * (LAG + 3)
            at_pool = [
                persist.tile([128, SH], bf16, name=f"at_{i}", tag="at", bufs=NAT)
                for i in range(NAT)
            ]
            outp_cur = {}  # seg index -> psum tile

            def emit_qk_exp(g):
                h, shi = segs[g // NTP]
                tp = g % NTP
                hp = h // 2
                ensure(("q", hp, shi * 2))
                ensure(("q", hp, shi * 2 + 1))
                ensure(("k", hp, (tp * 2) // 4))
                ensure(("k", hp, (tp * 2 + 1) // 4))
                qt, kt = qt_tiles[h], kt_tiles[h]
                s0 = shi * SH
                ats = []
                for j in range(2):
                    t = tp * 2 + j
                    sc_ps = ps.tile(
                        [128, SH], f32, name=f"sc_{g}_{j}", tag="sc", bufs=2
                    )
                    for scj in range(2):
                        dlo = scj * HD
                        nc.tensor.matmul(
                            sc_ps[:, scj * 512 : (scj + 1) * 512],
                            kt[dlo : dlo + HD, t * 128 : (t + 1) * 128],
                            qt[dlo : dlo + HD, s0 + scj * 512 : s0 + (scj + 1) * 512],
                            start=True,
                            stop=True,
                            skip_group_check=True,
                        )
                    at = at_pool[(2 * g + j) % NAT]
                    nc.scalar.activation(
                        at[:], sc_ps[:], AF.Exp, scale=0.125
                    )
                    ats.append(at)
                return ats

            at2_hist = {}

            def emit_pv(g):
                h, shi = segs[g // NTP]
                tp = g % NTP
                ensure(("v", tp * 2))
                ensure(("v", tp * 2 + 1))
                ats = at2_hist.pop(g)
                seg = g // NTP
                if tp == 0:
                    outp = ps.tile(
                        [VAUG, SH], f32, name=f"outp_{seg}", tag="outp", bufs=1
                    )
                    outp_cur[seg] = outp
                else:
                    outp = outp_cur[seg]
                for j in range(2):
                    t = tp * 2 + j
                    for scj in range(2):
                        nc.tensor.matmul(
                            outp[:, scj * 512 : (scj + 1) * 512],
                            v_aug[t][:, h * VAUG : (h + 1) * VAUG],
                            ats[j][:, scj * 512 : (scj + 1) * 512],
                            start=(tp == 0 and j == 0),
                            stop=(tp == NTP - 1 and j == 1),
                        )
                if tp == NTP - 1:
                    emit_epilogue(seg)

            def emit_epilogue(seg):
                h, shi = segs[seg]
                outp = outp_cur.pop(seg)
                oT = flow.tile([VW, SH], bf16, name=f"oT_{seg}", tag="oT", bufs=3)
                nc.vector.memset(oT[64:VW, :], 0.0)
                nc.vector.tensor_copy(oT[0:VAUG, :], outp[:])
                trb = flow.tile(
                    [128, (SH // 128) * VW],
                    bf16,
                    name=f"trb_{seg}",
                    tag="trb",
                    bufs=3,
                )
                trb3 = trb.rearrange("p (j c) -> p j c", j=SH // 128)
                # transpose in two halves so scales overlap the 2nd half
                nc.sync.dma_start_transpose(trb3[:, 0:4, :], oT[0:VW, 0:512])
                nc.sync.dma_start_transpose(trb3[:, 4:8, :], oT[0:VW, 512:1024])
                for j in range(SH // 128):
                    sb = shi * (SH // 128) + j
                    rs = flow.tile(
                        [128, 1], f32, name=f"rs_{seg}_{j}", tag="rs", bufs=8
                    )
                    nc.vector.reciprocal(rs[:], trb3[:, j, HD : HD + 1])
                    nc.vector.tensor_scalar_mul(
                        out_tiles[sb][:, h * HD : (h + 1) * HD],
                        trb3[:, j, 0:HD],
                        rs[:],
                    )
                    if h == 2:
                        # heads 0,1,2 columns final for this sb: ship early
                        eng = nc.sync if sb % 2 == 0 else nc.gpsimd
                        eng.dma_start(
                            out_d[sb * 128 : (sb + 1) * 128, 0 : 3 * HD],
                            out_tiles[sb][:, 0 : 3 * HD],
                        )
                    elif h == 3:
                        eng = nc.sync if sb % 2 == 0 else nc.gpsimd
                        eng.dma_start(
                            out_d[sb * 128 : (sb + 1) * 128, 3 * HD : C],
                            out_tiles[sb][:, 3 * HD : C],
                        )

            AUX_PER_ITER = 1
            pv_next = 0
            for g in range(n_iter):
                at2_hist[g] = emit_qk_exp(g)
                for _ in range(AUX_PER_ITER):
                    pump()
                # PV pacing: lag LAG while background projections still
                # pending (PE slack is scarce); then catch up to lag 2
                pending = bg_pending()
                lag = LAG if pending else (1 if g >= n_iter - 4 else 2)
                budget = 1 if pending else 2
                while budget > 0 and pv_next <= g - lag:
                    emit_pv(pv_next)
                    pv_next += 1
                    budget -= 1
            while pump():
                pass
            while pv_next < n_iter:
                emit_pv(pv_next)
                pv_next += 1

    nc.finalize()
    return nc


def _get_nc():
    if "nc" not in _CACHE:
        _CACHE["nc"] = _build_nc()
    return _CACHE["nc"]


def kernel(x, Wq, Wk, Wv):
    import ml_dtypes
    from concourse import bass_utils

    bf = ml_dtypes.bfloat16
    x = np.asarray(x, dtype=np.float32).astype(bf)
    Wq = np.asarray(Wq, dtype=np.float32).astype(bf)
    Wk = np.asarray(Wk, dtype=np.float32).astype(bf)
    Wv = np.asarray(Wv, dtype=np.float32).astype(bf)

    nc = _get_nc()
    in_maps = []
    for c in range(N_CORES):
        b = c // 4
        g = c % 4
        cols = slice(g * C, (g + 1) * C)
        blob = np.empty((128, 3 * NKT * C), dtype=Wq.dtype)
        for ti, W in enumerate((Wq, Wk, Wv)):
            Wc = W[:, cols]
            for kk in range(NKT):
                blob[:, (ti * NKT + kk) * C : (ti * NKT + kk + 1) * C] = Wc[
                    kk * 128 : (kk + 1) * 128, :
                ]
        in_maps.append(
            {
                "x": np.ascontiguousarray(
                    x[b].T.reshape(NKT, 128, S).transpose(1, 0, 2).reshape(128, NKT * S)
                ),
                "w": blob,
            }
        )

    res = bass_utils.run_bass_kernel_spmd(nc, in_maps, list(range(N_CORES)))
    _CACHE["last_results"] = res

    out = np.empty((B, S, RES), dtype=np.float32)
    for c in range(N_CORES):
        b = c // 4
        g = c % 4
        out[b, :, g * C : (g + 1) * C] = res.results[c]["out"]
    return out


# revision 33
# speedup vs baseline: 1.1826x; 1.1826x over previous
"""Multi-head attention (B=2, S=2048, RES=1024, H=16) on 8 NeuronCores.

Sharding: batch*heads across cores. Core c handles batch c//4 and heads
4*(c%4) .. 4*(c%4)+3 (column-sharded QKV weights). No cross-core comm.

Pipeline design (vs the naive phase-by-phase version):
  - Host pre-packs x into the transposed SBUF layout and all 24 weight
    k-chunks into one blob, so the input lands as 256 large descriptors
    (~18us at HBM bandwidth) instead of 4600 small ones.
  - Prologue projects QT/KT for heads 0,1 only (d-rows duplicated into
    partitions 64:128 so the two 512-wide QK matmuls of a score tile
    run as concurrent PE row tiles); first exp fires ~20us in.
  - Main loop over segments (head, s-half) x t-pairs: QK scores (PSUM)
    -> exp on ACT (bf16, scale=1/8) -> PV (full-K bf16, M=66 with a
    ones column so softmax sums ride along).  PV lags the exp stream by
    LAG iterations, tapering to 2 once background work drains.
  - V projection and heads 2,3 QT/KT projection are "ensure" units:
    consumers force-emit their producers, so any pacing of the
    background pump is correct by construction (Tile engines execute in
    emission order - a consumer emitted before its producer silently
    reads stale data).
  - Per-(head,s-half) epilogue (PSUM drain, xbar detranspose in two
    halves, 1/sums scale) is inline; out DMAs for head-columns 0:192
    ship one segment early so the tail only writes the last 64 columns.
PE (projections + QK + PV, ~150us) and ACT (16.8M exps, ~131-160us
clock-dependent) are both near-critical; PSUM: 2 score tiles (4 banks)
+ PV accumulator (2) + projection staging (2).
"""

import sys

if "/opt/trn_rl_repo" not in sys.path:
    sys.path.insert(0, "/opt/trn_rl_repo")

import numpy as np

B = 2
S = 2048
RES = 1024
HEADS = 16
HD = 64
N_CORES = 8
HPC = 4
C = HPC * HD  # 256
K = RES
NKT = K // 128  # 8
NST = S // 128  # 16 t/s blocks
NTP = NST // 2  # 8 t-pairs
SH = 1024  # s-half
VW = 80  # oT row count for xbar transpose (64 d + ones + pad)
VAUG = 66  # per-head width in v_aug (64 d + ones col + pad)
LAG = 10  # PV lag in (head, s-half, t-pair) iterations

_CACHE: dict = {}


def _build_nc():
    import concourse.mybir as mybir
    import concourse.tile as tile
    from concourse import bacc

    f32 = mybir.dt.float32
    bf16 = mybir.dt.bfloat16
    AF = mybir.ActivationFunctionType

    nc = bacc.Bacc(None)
    x_in = nc.dram_tensor("x", [128, NKT * S], bf16, kind="ExternalInput")  # host-packed xT layout
    w_in = nc.dram_tensor("w", [128, 3 * NKT * C], bf16, kind="ExternalInput")  # host-packed wq|wk|wv chunks
    out_d = nc.dram_tensor("out", [S, C], f32, kind="ExternalOutput")

    with tile.TileContext(nc) as tc:
        with (
            tc.tile_pool(name="persist", bufs=1) as persist,
            tc.tile_pool(name="flow", bufs=2) as flow,
            tc.tile_pool(name="ps", bufs=1, space="PSUM") as ps,
        ):
            zeros128 = persist.tile([128, 128], bf16)
            nc.vector.memset(zeros128[:], 0.0)

            # ACT table preload: tiny exp early so the ~2.7us table load
            # overlaps the initial DMA wait
            warmup_exp = persist.tile([1, 16], bf16)
            nc.scalar.activation(
                warmup_exp[:], zeros128[0:1, 0:16], AF.Exp, scale=0.125
            )

            # ---- input DMAs (one packed weight blob: 128 x 12KB
            # descriptors instead of 3072 x 512B; xT right behind) ----
            w_sb = persist.tile([128, 3 * NKT, C], bf16, name="w_sb")
            nc.gpsimd.dma_start(
                w_sb.rearrange("p a b -> p (a b)"), w_in[:, :]
            )
            wq_t = [w_sb[:, 0 * NKT + kk, :] for kk in range(NKT)]
            wk_t = [w_sb[:, 1 * NKT + kk, :] for kk in range(NKT)]
            wv_t = [w_sb[:, 2 * NKT + kk, :] for kk in range(NKT)]
            # two xT tiles (k-chunks 0-3 / 4-7): Tile deps are per-tile,
            # so first-half projections start as soon as half 0 lands
            HK = NKT // 2
            xTa = persist.tile([128, HK * S], bf16, name="xTa")
            xTb = persist.tile([128, HK * S], bf16, name="xTb")
            nc.sync.dma_start(xTa[:], x_in[:, 0 : HK * S])
            nc.sync.dma_start(xTb[:], x_in[:, HK * S : NKT * S])
            xTa3 = xTa.rearrange("p (k s) -> p k s", k=HK)
            xTb3 = xTb.rearrange("p (k s) -> p k s", k=HK)

            def xchunk(kk, sl):
                if kk < HK:
                    return xTa3[:, kk, sl]
                return xTb3[:, kk - HK, sl]

            # per-head Q^T/K^T, d-rows duplicated into partitions 64:128
            qt_tiles = [
                persist.tile([128, S], bf16, name=f"qt_{h}", tag="qt", bufs=HPC)
                for h in range(HPC)
            ]
            kt_tiles = [
                persist.tile([128, S], bf16, name=f"kt_{h}", tag="kt", bufs=HPC)
                for h in range(HPC)
            ]

            # bf16 V tiles per t-block: [128, 4*VAUG]; per head: V cols
            # 0:64, ones at 64, zero pad at 65
            v_aug = []
            for st in range(NST):
                va = persist.tile(
                    [128, HPC * VAUG], bf16, name=f"va_{st}", tag="va", bufs=NST
                )
                nc.vector.memset(va[:], 0.0)
                nc.vector.memset(
                    va.rearrange("p (h w) -> p h w", h=HPC)[:, :, HD : HD + 1],
                    1.0,
                )
                v_aug.append(va)

            out_tiles = [
                persist.tile([128, C], f32, name=f"out_{sb}", tag="ot", bufs=NST)
                for sb in range(NST)
            ]

            # PSUM: sc 2x[128,1024] (4 banks) + outp 1x[80,1024] (2 banks)
            # + pp 2x[128,512] (2 banks) = 8 banks
            pp_warm = ps.tile([128, 512], f32, name="pp_warm", tag="pp", bufs=2)
            for w in range(24):
                nc.tensor.matmul(
                    pp_warm[:, 0:128],
                    zeros128[:],
                    zeros128[:],
                    start=True,
                    stop=True,
                    skip_group_check=True,
                )

            # ---------- background work units (PE-stream filler) ----------
            proj_qk_store = {}

            def proj_qk_unit(w_t, dsts, hp, sc, half):
                # Row-tiled halves (k rows 0:64 / 64:128 on PE row groups
                # 0/64) run concurrently and hide each other's LDWEIGHTS;
                # DVE sums the two partial products at staging time.
                def emit():
                    key = f"{id(dsts)}_{hp}_{sc}"
                    if half == 0:
                        pp = ps.tile(
                            [128, 512], f32, name=f"pp_{key}", tag="pp", bufs=2
                        )
                        proj_qk_store[key] = pp
                    else:
                        pp = proj_qk_store.pop(key)
                    for kk in range(half * 4, half * 4 + 4):
                        nc.tensor.matmul(
                            pp[:],
                            w_t[kk][:, hp * 128 : (hp + 1) * 128],
                            xchunk(kk, slice(sc * 512, (sc + 1) * 512)),
                            start=(kk == 0),
                            stop=(kk == NKT - 1),
                        )
                    if half == 0:
                        return
                    stg = flow.tile(
                        [128, 512], bf16, name=f"stg_{key}", tag="stg", bufs=2
                    )
                    nc.vector.tensor_copy(stg[:], pp[:])
                    cols = slice(sc * 512, (sc + 1) * 512)
                    for hh in range(2):
                        h = hp * 2 + hh
                        nc.vector.tensor_copy(
                            dsts[h][0:HD, cols], stg[hh * HD : (hh + 1) * HD, :]
                        )
                        nc.vector.tensor_copy(
                            dsts[h][HD:128, cols], stg[hh * HD : (hh + 1) * HD, :]
                        )
                return emit

            def vproj_unit(st, half):
                # V projection for s-block st, row-tiled halves like above;
                # DVE sums partials straight into the strided v_aug layout
                def emit():
                    key = f"v_{st}"
                    if half == 0:
                        vp = ps.tile(
                            [128, 512], f32, name=f"vp_{st}", tag="pp", bufs=2
                        )
                        proj_qk_store[key] = vp
                    else:
                        vp = proj_qk_store.pop(key)
                    for kk in range(half * 4, half * 4 + 4):
                        nc.tensor.matmul(
                            vp[:, 0:C],
                            xchunk(kk, slice(st * 128, (st + 1) * 128)),
                            wv_t[kk],
                            start=(kk == 0),
                            stop=(kk == NKT - 1),
                        )
                    if half == 0:
                        return
                    dst = v_aug[st].rearrange("p (h w) -> p h w", h=HPC)[
                        :, :, 0:HD
                    ]
                    nc.vector.tensor_copy(
                        dst, vp[:, 0:C].rearrange("p (h d) -> p h d", h=HPC)
                    )
                return emit

            # Unit registry: consumers force-emit ("ensure") their producer
            # units, so emission order is correct by construction no matter
            # how the background pump is paced.
            units = {}
            for hp in range(2):
                for sc in range(4):
                    units[("q", hp, sc)] = [
                        proj_qk_unit(wq_t, qt_tiles, hp, sc, half)
                        for half in range(2)
                    ]
                    units[("k", hp, sc)] = [
                        proj_qk_unit(wk_t, kt_tiles, hp, sc, half)
                        for half in range(2)
                    ]
            for st in range(NST):
                units[("v", st)] = [vproj_unit(st, half) for half in range(2)]
            emitted = set()

            def ensure(key):
                if key in emitted:
                    return
                emitted.add(key)
                for fn in units[key]:
                    fn()

            # background order, in deadline order
            bg_order = (
                [("k", 0, sc) for sc in range(4)]
                + [("q", 0, 2), ("q", 0, 3)]
                + [("v", st) for st in range(NST)]
                + [("q", 1, 0), ("q", 1, 1)]
                + [("k", 1, sc) for sc in range(4)]
                + [("q", 1, 2), ("q", 1, 3)]
                + [("q", 0, 0), ("q", 0, 1)]
            )
            bg_pos = [0]

            def pump():
                while bg_pos[0] < len(bg_order):
                    key = bg_order[bg_pos[0]]
                    if key not in emitted:
                        ensure(key)
                        return True
                    bg_pos[0] += 1
                return False

            def bg_pending():
                return any(k not in emitted for k in bg_order)

            # prologue: what iteration 0 needs
            ensure(("q", 0, 0))
            ensure(("q", 0, 1))
            ensure(("k", 0, 0))

            # ---------- main attention pipeline ----------
            segs = [(0, 0), (1, 0), (0, 1), (1, 1),
                    (2, 0), (3, 0), (2, 1), (3, 1)]
            n_iter = len(segs) * NTP  # 64
            NAT = 2 * (LAG + 3)
            at_pool = [
                persist.tile([128, SH], bf16, name=f"at_{i}", tag="at", bufs=NAT)
                for i in range(NAT)
            ]
            outp_cur = {}  # seg index -> psum tile

            def emit_qk_exp(g):
                h, shi = segs[g // NTP]
                tp = g % NTP
                hp = h // 2
                ensure(("q", hp, shi * 2))
                ensure(("q", hp, shi * 2 + 1))
                ensure(("k", hp, (tp * 2) // 4))
                ensure(("k", hp, (tp * 2 + 1) // 4))
                qt, kt = qt_tiles[h], kt_tiles[h]
                s0 = shi * SH
                ats = []
                for j in range(2):
                    t = tp * 2 + j
                    sc_ps = ps.tile(
                        [128, SH], f32, name=f"sc_{g}_{j}", tag="sc", bufs=2
                    )
                    for scj in range(2):
                        dlo = scj * HD
                        nc.tensor.matmul(
                            sc_ps[:, scj * 512 : (scj + 1) * 512],
                            kt[dlo : dlo + HD, t * 128 : (t + 1) * 128],
                            qt[dlo : dlo + HD, s0 + scj * 512 : s0 + (scj + 1) * 512],
                            start=True,
                            stop=True,
                            skip_group_check=True,
                        )
                    at = at_pool[(2 * g + j) % NAT]
                    nc.scalar.activation(
                        at[:], sc_ps[:], AF.Exp, scale=0.125
                    )
                    ats.append(at)
                return ats

            at2_hist = {}

            def emit_pv(g):
                h, shi = segs[g // NTP]
                tp = g % NTP
                ensure(("v", tp * 2))
                ensure(("v", tp * 2 + 1))
                ats = at2_hist.pop(g)
                seg = g // NTP
                if tp == 0:
                    outp = ps.tile(
                        [VAUG, SH], f32, name=f"outp_{seg}", tag="outp", bufs=1
                    )
                    outp_cur[seg] = outp
                else:
                    outp = outp_cur[seg]
                for j in range(2):
                    t = tp * 2 + j
                    for scj in range(2):
                        nc.tensor.matmul(
                            outp[:, scj * 512 : (scj + 1) * 512],
                            v_aug[t][:, h * VAUG : (h + 1) * VAUG],
                            ats[j][:, scj * 512 : (scj + 1) * 512],
                            start=(tp == 0 and j == 0),
                            stop=(tp == NTP - 1 and j == 1),
                        )
                if tp == NTP - 1:
                    emit_epilogue(seg)

            def emit_epilogue(seg):
                h, shi = segs[seg]
                outp = outp_cur.pop(seg)
                oT = flow.tile([VW, SH], bf16, name=f"oT_{seg}", tag="oT", bufs=3)
                nc.vector.memset(oT[64:VW, :], 0.0)
                nc.vector.tensor_copy(oT[0:VAUG, :], outp[:])
                trb = flow.tile(
                    [128, (SH // 128) * VW],
                    bf16,
                    name=f"trb_{seg}",
                    tag="trb",
                    bufs=3,
                )
                trb3 = trb.rearrange("p (j c) -> p j c", j=SH // 128)
                # transpose in two halves so scales overlap the 2nd half
                nc.sync.dma_start_transpose(trb3[:, 0:4, :], oT[0:VW, 0:512])
                nc.sync.dma_start_transpose(trb3[:, 4:8, :], oT[0:VW, 512:1024])
                for j in range(SH // 128):
                    sb = shi * (SH // 128) + j
                    rs = flow.tile(
                        [128, 1], f32, name=f"rs_{seg}_{j}", tag="rs", bufs=8
                    )
                    nc.vector.reciprocal(rs[:], trb3[:, j, HD : HD + 1])
                    nc.vector.tensor_scalar_mul(
                        out_tiles[sb][:, h * HD : (h + 1) * HD],
                        trb3[:, j, 0:HD],
                        rs[:],
                    )
                    if h == 2:
                        # heads 0,1,2 columns final for this sb: ship early
                        eng = nc.sync if sb % 2 == 0 else nc.gpsimd
                        eng.dma_start(
                            out_d[sb * 128 : (sb + 1) * 128, 0 : 3 * HD],
                            out_tiles[sb][:, 0 : 3 * HD],
                        )
                    elif h == 3:
                        eng = nc.sync if sb % 2 == 0 else nc.gpsimd
                        eng.dma_start(
                            out_d[sb * 128 : (sb + 1) * 128, 3 * HD : C],
                            out_tiles[sb][:, 3 * HD : C],
                        )

            AUX_PER_ITER = 1
            pv_next = 0
            for g in range(n_iter):
                at2_hist[g] = emit_qk_exp(g)
                for _ in range(AUX_PER_ITER):
                    pump()
                # PV pacing: lag LAG while background projections still
                # pending (PE slack is scarce); then catch up to lag 2
                pending = bg_pending()
                lag = LAG if pending else 2
                budget = 1 if pending else 2
                while budget > 0 and pv_next <= g - lag:
                    emit_pv(pv_next)
                    pv_next += 1
                    budget -= 1
            while pump():
                pass
            while pv_next < n_iter:
                emit_pv(pv_next)
                pv_next += 1

    nc.finalize()
    return nc


def _get_nc():
    if "nc" not in _CACHE:
        _CACHE["nc"] = _build_nc()
    return _CACHE["nc"]


def kernel(x, Wq, Wk, Wv):
    import ml_dtypes
    from concourse import bass_utils

    bf = ml_dtypes.bfloat16
    x = np.asarray(x, dtype=np.float32).astype(bf)
    Wq = np.asarray(Wq, dtype=np.float32).astype(bf)
    Wk = np.asarray(Wk, dtype=np.float32).astype(bf)
    Wv = np.asarray(Wv, dtype=np.float32).astype(bf)

    nc = _get_nc()
    in_maps = []
    for c in range(N_CORES):
        b = c // 4
        g = c % 4
        cols = slice(g * C, (g + 1) * C)
        blob = np.empty((128, 3 * NKT * C), dtype=Wq.dtype)
        for ti, W in enumerate((Wq, Wk, Wv)):
            Wc = W[:, cols]
            for kk in range(NKT):
                blob[:, (ti * NKT + kk) * C : (ti * NKT + kk + 1) * C] = Wc[
                    kk * 128 : (kk + 1) * 128, :
                ]
        in_maps.append(
            {
                "x": np.ascontiguousarray(
                    x[b].T.reshape(NKT, 128, S).transpose(1, 0, 2).reshape(128, NKT * S)
                ),
                "w": blob,
            }
        )

    res = bass_utils.run_bass_kernel_spmd(nc, in_maps, list(range(N_CORES)))
    _CACHE["last_results"] = res

    out = np.empty((B, S, RES), dtype=np.float32)
    for c in range(N_CORES):
        b = c // 4
        g = c % 4
        out[b, :, g * C : (g + 1) * C] = res.results[c]["out"]
    return out
